# revision 1
# baseline (speedup 1.0000x reference)
"""Multi-head attention (B=2, S=2048, D=1024, H=16, Dh=64) on 8 Trainium2
NeuronCores.

Sharding: data-parallel over batch (2 groups of 4 cores) x tensor-parallel
over heads (4 heads per core; Wq/Wk/Wv column-sharded, Wo row-sharded).
Per core (batch b, head-group g):
    QT/KT = (x @ Wq_g + bq_g)^T per head, zero-padded to 96 rows (a K<=64
        contraction lowers to the half-rate tiled matmul mode; K=96 rounds
        up to the full 128-row mode at full speed), bf16.
    per head h: expS^T = exp(scores^T/8 + maskbias) via ACT straight out of
        PSUM (no max-subtraction: scores ~ N(0,1) at this problem's scale).
    CT_h = Vaug_h^T @ expS^T with Vaug carrying a ones column, so PV psum
        row 64 accumulates the softmax denominator for free.
    normalize: PE-broadcast the rowsum, reciprocal_approx_fast on 128
        lanes, multiply during CT eviction into packed [128 x S] bf16 tiles
        (head pairs), so the output projection contracts K=128.
    O_partial = sum_h CT_h_norm @ Wo_h.
Host sums the 4 partials per batch and adds bo.

Projections/PV run in float32r (tf32-like, full PE rate at K>=65);
scores + output projection in bf16. End-to-end rel err vs the fp32
reference ~1e-3.
"""

import os
import sys

for _p in ("/opt/trn_rl_repo", "/root/.axon_site/_ro/trn_rl_repo"):
    if os.path.isdir(_p) and _p not in sys.path:
        sys.path.insert(0, _p)

import numpy as np

import concourse.bass as bass
import concourse.mybir as mybir
from concourse import bass_utils
from concourse.tile import TileContext
from concourse.vector_clock import ScopedClock

# ---------------------------------------------------------------------------
# Walrus in this container rejects instructions carrying more than one sync
# wait. Tile's scheduler freely emits several waits per instruction, so split
# the extras onto preceding same-engine nops (engines execute in order, so a
# nop completing its wait guarantees the condition for the next instruction).
# ---------------------------------------------------------------------------

_ENGINE_BUILDER = {
    mybir.EngineType.PE: "tensor",
    mybir.EngineType.DVE: "vector",
    mybir.EngineType.Activation: "scalar",
    mybir.EngineType.Pool: "gpsimd",
    mybir.EngineType.SP: "sync",
}


def _make_nop_with_wait(nc, engine, wait):
    builder = getattr(nc, _ENGINE_BUILDER[engine])
    bi = builder.nop(nofuse=True, hint="split_wait")
    inst = bi.ins
    for f in nc.m.functions:
        for b in f.blocks:
            il = b.instructions
            if il and il[-1] is inst:
                il.pop()
    si = inst.sync_info
    if si is None:
        inst.sync_info = mybir.SyncInfo(on_wait=[wait], on_update=[])
    else:
        si.on_wait = [wait]
    return inst


def split_sync_waits(nc, cap=1):
    for f in nc.m.functions:
        for b in f.blocks:
            il = b.instructions
            out = []
            changed = False
            for inst in il:
                si = inst.sync_info
                waits = list(si.on_wait) if si is not None and si.on_wait else []
                if len(waits) > cap and inst.engine in _ENGINE_BUILDER:
                    si.on_wait = waits[-cap:]
                    for w in waits[:-cap]:
                        out.append(_make_nop_with_wait(nc, inst.engine, w))
                    changed = True
                out.append(inst)
            if changed:
                b.instructions = out


class PatchedTileContext(TileContext):
    def _drain_and_barrier(self, tick_clock, wait_clock):
        drain_inst = self.nc.sync.drain()
        wait_clock.add_sem_waits(
            drain_inst.ins, ScopedClock({None: tick_clock.global_clock})
        )
        si = drain_inst.ins.sync_info
        waits = list(si.on_wait or [])
        if len(waits) > 1:
            si.on_wait = waits[:1]
            for i in range(1, len(waits)):
                extra = self.nc.sync.drain()
                esi = extra.ins.sync_info
                if esi is None:
                    extra.ins.sync_info = mybir.SyncInfo(
                        on_wait=[waits[i]], on_update=[]
                    )
                else:
                    esi.on_wait = [waits[i]]
        self.nc.all_engine_barrier()
        assert self.sems is not None
        popped = self.nc._tile_sem_poison_stack.pop()
        assert popped is self._sem_poison
        self.nc.clear_and_free_semaphores(list(self.sems.allocated().values()))
        self.nc.all_engine_barrier()

    def __exit__(self, *args):
        r = super().__exit__(*args)
        split_sync_waits(self.nc, cap=1)
        return r


# ---------------------------------------------------------------------------
# Problem shapes (hardcoded per the harness contract).
# ---------------------------------------------------------------------------

B, S, D = 2, 2048, 1024
NUM_HEADS, HEAD_DIM = 16, 64
N_CORES = 8
HPC = 4                     # heads per core
C = HPC * HEAD_DIM          # 256 projection columns per core
F32 = mybir.dt.float32
F32R = mybir.dt.float32r
BF16 = mybir.dt.bfloat16
SCALE = 1.0 / np.sqrt(HEAD_DIM)   # 0.125
MASK_NEG = -30.0            # exp(-30 + smax) ~ 0 for this problem's score range
KPAD = 96                   # head-dim padded so matmul uses the full-rate mode

SD = S // 512               # 4 chunks of 512 along S
ST = S // 128               # 16 tiles of 128 along S
DT = D // 128               # 8 tiles of 128 along D


def _build_nc():
    nc = bass.Bass(trn_type="TRN2", target_bir_lowering=False, debug=False)

    xT = nc.dram_tensor("xT", [D, S], F32R, kind="ExternalInput")
    wq = nc.dram_tensor("wq", [D, C], F32R, kind="ExternalInput")
    wk = nc.dram_tensor("wk", [D, C], F32R, kind="ExternalInput")
    wv = nc.dram_tensor("wv", [D, C], F32R, kind="ExternalInput")
    wo = nc.dram_tensor("wo", [2, 128, D], BF16, kind="ExternalInput")
    bqr = nc.dram_tensor("bqr", [2, 128], F32, kind="ExternalInput")
    bkr = nc.dram_tensor("bkr", [2, 128], F32, kind="ExternalInput")
    bvr = nc.dram_tensor("bvr", [1, C], F32R, kind="ExternalInput")
    maskb = nc.dram_tensor("maskb", [ST, 128], F32, kind="ExternalInput")
    o = nc.dram_tensor("o", [S, D], F32, kind="ExternalOutput")

    with PatchedTileContext(nc) as tc, nc.allow_low_precision(
        reason="f32r/bf16 compute; verified end-to-end vs reference"
    ):
        Exp = mybir.ActivationFunctionType.Exp
        with tc.tile_pool(name="const", bufs=1) as constp, \
             tc.tile_pool(name="qk", bufs=1) as qkp, \
             tc.tile_pool(name="vt", bufs=1) as vtp:

            # ---- constants resident for the whole kernel ----
            wop = [constp.tile([128, D], BF16, name=f"wop{i}", tag=f"wop{i}")
                   for i in range(2)]
            for i in range(2):
                nc.sync.dma_start(wop[i][:], wo[i, :, :])
            bq_sb = constp.tile([128, 2], F32, name="bq_sb")
            bk_sb = constp.tile([128, 2], F32, name="bk_sb")
            bv_sb = constp.tile([1, C], F32R, name="bv_sb")
            maskb_sb = constp.tile([128, ST], F32, name="maskb_sb")
            nc.sync.dma_start(bq_sb[:], bqr.ap().rearrange("t p -> p t"))
            nc.sync.dma_start(bk_sb[:], bkr.ap().rearrange("t p -> p t"))
            nc.sync.dma_start(bv_sb[:], bvr[:, :])
            nc.sync.dma_start(maskb_sb[:], maskb.ap().rearrange("t p -> p t"))
            ones_f32 = constp.tile([128, 128], F32, name="ones_f32")
            nc.vector.memset(ones_f32[:], 1.0)
            ones_r = constp.tile([1, 128], F32R, name="ones_r")
            nc.vector.tensor_copy(ones_r[:], ones_f32[0:1, :])
            ones_bf = constp.tile([128, HPC], BF16, name="ones_bf")
            nc.vector.memset(ones_bf[:], 1.0)

            # ---- persistent activations ----
            # per-head QT/KT, rows 64..95 zero so scores contract K=96
            qth = [qkp.tile([KPAD, S], BF16, name=f"qth{h}", tag=f"qth{h}")
                   for h in range(HPC)]
            kth = [qkp.tile([KPAD, S], BF16, name=f"kth{h}", tag=f"kth{h}")
                   for h in range(HPC)]
            for h in range(HPC):
                nc.vector.memset(qth[h][HEAD_DIM:KPAD, :], 0.0)
                nc.vector.memset(kth[h][HEAD_DIM:KPAD, :], 0.0)
            vt = [vtp.tile([128, HPC * 65], BF16, name=f"vt{s}", tag=f"vt{s}")
                  for s in range(ST)]

            # =============== phase 1: projections ===============
            with tc.tile_pool(name="xtp", bufs=1) as xtp, \
                 tc.tile_pool(name="wproj", bufs=1) as wpp, \
                 tc.tile_pool(name="ps_qk", bufs=4, space="PSUM") as ps_qkp, \
                 tc.tile_pool(name="ps_v", bufs=2, space="PSUM") as ps_vp:

                xt = [xtp.tile([128, S], F32R, name=f"xt{d}", tag=f"xt{d}")
                      for d in range(DT)]
                wqt = [wpp.tile([128, C], F32R, name=f"wqt{d}", tag=f"wqt{d}")
                       for d in range(DT)]
                wkt = [wpp.tile([128, C], F32R, name=f"wkt{d}", tag=f"wkt{d}")
                       for d in range(DT)]
                wvt = [wpp.tile([128, C], F32R, name=f"wvt{d}", tag=f"wvt{d}")
                       for d in range(DT)]
                for d in range(DT):
                    nc.sync.dma_start(wqt[d][:], wq[d * 128:(d + 1) * 128, :])
                    nc.sync.dma_start(wkt[d][:], wk[d * 128:(d + 1) * 128, :])
                    nc.sync.dma_start(wvt[d][:], wv[d * 128:(d + 1) * 128, :])
                    nc.sync.dma_start(xt[d][:], xT[d * 128:(d + 1) * 128, :])

                # QT / KT: psum [c-tile 128, s-chunk 512] -> split per head
                for wt, dst, bias in ((wqt, qth, bq_sb), (wkt, kth, bk_sb)):
                    for ci in range(2):
                        ps = [ps_qkp.tile([128, 512], F32, name=f"psqk{ci}{s4}",
                                          tag="psqk")
                              for s4 in range(SD)]
                        for d in range(DT):
                            for s4 in range(SD):
                                nc.tensor.matmul(
                                    ps[s4][:],
                                    wt[d][:, ci * 128:(ci + 1) * 128],
                                    xt[d][:, s4 * 512:(s4 + 1) * 512],
                                    start=(d == 0), stop=(d == DT - 1),
                                )
                        for s4 in range(SD):
                            sl = slice(s4 * 512, (s4 + 1) * 512)
                            nc.vector.tensor_scalar_add(
                                dst[2 * ci][0:HEAD_DIM, sl],
                                ps[s4][0:HEAD_DIM, :], bias[0:HEAD_DIM, ci:ci + 1],
                            )
                            nc.vector.tensor_scalar_add(
                                dst[2 * ci + 1][0:HEAD_DIM, sl],
                                ps[s4][HEAD_DIM:128, :], bias[HEAD_DIM:128, ci:ci + 1],
                            )

                # V: out [s-tile 128, 256] natural layout; ones-row matmul
                # adds bv; evict strided into Vaug (ones col per head).
                for s in range(ST):
                    psv = ps_vp.tile([128, C], F32, name=f"psv{s}", tag="psv")
                    for d in range(DT):
                        nc.tensor.matmul(
                            psv[:], xt[d][:, s * 128:(s + 1) * 128], wvt[d][:],
                            start=(d == 0), stop=False,
                        )
                    nc.tensor.matmul(
                        psv[:], ones_r[:, 0:128], bv_sb[:],
                        start=False, stop=True,
                    )
                    dstv = vt[s][:].rearrange("p (h e) -> p h e", e=65)
                    nc.vector.tensor_copy(
                        dstv[:, :, 0:64],
                        psv[:].rearrange("p (h d) -> p h d", h=HPC),
                    )
                    nc.vector.tensor_copy(
                        dstv[:, :, 64:65],
                        ones_bf[:, :].rearrange("p (h e) -> p h e", e=1),
                    )

            # =============== phase 2: attention ===============
            with tc.tile_pool(name="ctp", bufs=1) as ctp, \
                 tc.tile_pool(name="rsp", bufs=2) as rsp:
                ctpk = [ctp.tile([128, S], BF16, name=f"ctp{i}", tag=f"ctp{i}")
                        for i in range(2)]

                with tc.tile_pool(name="etp", bufs=6) as etp, \
                     tc.tile_pool(name="bcp", bufs=4) as bcp, \
                     tc.tile_pool(name="pss", bufs=3, space="PSUM") as pssp, \
                     tc.tile_pool(name="pv", bufs=1, space="PSUM") as pvp, \
                     tc.tile_pool(name="pbc", bufs=1, space="PSUM") as pbcp:

                    def head_tail(h, pv):
                        """normalize head h's context out of its PV psum."""
                        rs = rsp.tile([1, S], F32R, name=f"rs{h}", tag="rs")
                        nc.vector.tensor_copy(rs[:], pv[64:65, :])
                        i, hi = h // 2, h % 2
                        rows = slice(64 * hi, 64 * hi + 64)
                        for q4 in range(SD):
                            sl = slice(q4 * 512, (q4 + 1) * 512)
                            pbc = pbcp.tile([128, 512], F32,
                                            name=f"pbc{h}{q4}", tag="pbc")
                            nc.tensor.matmul(
                                pbc[:], ones_r[:, :], rs[:, sl],
                                start=True, stop=True,
                            )
                            bc = bcp.tile([128, 512], F32,
                                          name=f"bc{h}{q4}", tag="bc")
                            nc.vector.reciprocal(bc[rows, :], pbc[rows, :])
                            nc.vector.tensor_mul(
                                ctpk[i][rows, sl], pv[0:64, sl], bc[rows, :],
                            )

                    prev = None  # (head, pv) awaiting tail
                    for h in range(HPC):
                        pv = pvp.tile([65, S], F32, name=f"pv{h}", tag="pv")
                        ets = []
                        for k in range(ST):
                            et = etp.tile([128, S], BF16, name=f"et{h}_{k}",
                                          tag="et")
                            for q4 in range(SD):
                                pss = pssp.tile([128, 512], F32,
                                                name=f"pss{h}_{k}_{q4}", tag="pss")
                                nc.tensor.matmul(
                                    pss[:],
                                    kth[h][:, k * 128:(k + 1) * 128],
                                    qth[h][:, q4 * 512:(q4 + 1) * 512],
                                    start=True, stop=True,
                                )
                                nc.scalar.activation(
                                    et[:, q4 * 512:(q4 + 1) * 512], pss[:],
                                    Exp, bias=maskb_sb[:, k:k + 1], scale=SCALE,
                                )
                            ets.append(et)
                            if prev is not None and k == 2:
                                head_tail(*prev)
                                prev = None
                            if k > 0:
                                kp = k - 1
                                for q4 in range(SD):
                                    nc.tensor.matmul(
                                        pv[:, q4 * 512:(q4 + 1) * 512],
                                        vt[kp][:, 65 * h:65 * h + 65],
                                        ets[kp][:, q4 * 512:(q4 + 1) * 512],
                                        start=(kp == 0), stop=False,
                                    )
                        for q4 in range(SD):
                            nc.tensor.matmul(
                                pv[:, q4 * 512:(q4 + 1) * 512],
                                vt[ST - 1][:, 65 * h:65 * h + 65],
                                ets[ST - 1][:, q4 * 512:(q4 + 1) * 512],
                                start=False, stop=True,
                            )
                        prev = (h, pv)
                    head_tail(*prev)

                # =============== phase 3: output projection ===============
                with tc.tile_pool(name="obp", bufs=3) as obp, \
                     tc.tile_pool(name="po", bufs=3, space="PSUM") as pop:
                    for s in range(ST):
                        for n2 in range(2):
                            p_o = pop.tile([128, 512], F32, name=f"po{s}{n2}",
                                           tag="po")
                            for i in range(2):
                                nc.tensor.matmul(
                                    p_o[:],
                                    ctpk[i][:, s * 128:(s + 1) * 128],
                                    wop[i][:, n2 * 512:(n2 + 1) * 512],
                                    start=(i == 0), stop=(i == 1),
                                )
                            ob = obp.tile([128, 512], F32, name=f"ob{s}{n2}",
                                          tag="ob")
                            nc.vector.tensor_copy(ob[:], p_o[:])
                            nc.sync.dma_start(
                                o[s * 128:(s + 1) * 128,
                                  n2 * 512:(n2 + 1) * 512], ob[:],
                            )
    return nc


_NC_CACHE = {}


def get_nc():
    if "nc" not in _NC_CACHE:
        _NC_CACHE["nc"] = _build_nc()
    return _NC_CACHE["nc"]


def _in_maps(x, attention_mask, Wq, bq, Wk, bk, Wv, bv, Wo, bo):
    import ml_dtypes
    f32 = np.float32
    maps = []
    xTb = [np.ascontiguousarray(np.asarray(x[b], f32).T) for b in range(B)]
    maskbb = [
        ((np.asarray(attention_mask[b]).astype(f32) - 1.0) * -MASK_NEG
         ).reshape(ST, 128).astype(f32)
        for b in range(B)
    ]
    Wq, Wk, Wv, Wo = (np.asarray(a, f32) for a in (Wq, Wk, Wv, Wo))
    bq, bk, bv = (np.asarray(a, f32) for a in (bq, bk, bv))
    for c in range(N_CORES):
        b, g = divmod(c, N_CORES // B)
        cs = slice(g * C, (g + 1) * C)
        maps.append({
            "xT": xTb[b],
            "wq": np.ascontiguousarray(Wq[:, cs]),
            "wk": np.ascontiguousarray(Wk[:, cs]),
            "wv": np.ascontiguousarray(Wv[:, cs]),
            "wo": np.ascontiguousarray(Wo[cs, :]).reshape(2, 128, D)
                    .astype(ml_dtypes.bfloat16),
            "bqr": np.ascontiguousarray(bq[cs]).reshape(2, 128),
            "bkr": np.ascontiguousarray(bk[cs]).reshape(2, 128),
            "bvr": np.ascontiguousarray(bv[cs]).reshape(1, C),
            "maskb": maskbb[b],
        })
    return maps


def run(trace=False, **inputs):
    nc = get_nc()
    maps = _in_maps(**inputs)
    res = bass_utils.run_bass_kernel_spmd(
        nc, maps, core_ids=list(range(N_CORES)), trace=trace
    )
    bo = np.asarray(inputs["bo"], np.float32)
    out = np.empty((B, S, D), np.float32)
    for b in range(B):
        acc = res.results[b * 4 + 0]["o"].astype(np.float32).copy()
        for g in range(1, N_CORES // B):
            acc += res.results[b * 4 + g]["o"]
        out[b] = acc + bo[None, :]
    return out, res


def kernel(**inputs):
    out, _ = run(trace=False, **inputs)
    return out

